# revision 48
# baseline (speedup 1.0000x reference)
"""Multi-head self-attention (RoPE, causal) Bass kernel for 8 TRN2 NeuronCores.

Sharding: tensor-parallel over heads for QKV+attention (2 heads/core),
chunked AllToAll, then token-parallel O-projection (512 tokens/core).

bf16 data path (fp32 PSUM accumulation + fp32 softmax statistics):
  xt/wq/wk/wv/wo/cos/sin/qT/kT/aoT/a2a payload are bf16 (FWL weight loads,
  half the DMA + collective bytes); at/v are f32r (full-rate exp on ACT,
  1 cyc/row matmul at N>=256). Measured rel err ~6.8e-3 (gate 2e-2).

Layouts (per core):
  qT/kT:    [128 part = 2 heads x 64 dk, t] bf16 (RoPE'd projections)
  scoresT:  [128 part = k-tile, q free] PSUM f32 (softmax sum via ones-row)
  v_sb:     [128 part = k-tile tokens, 130] f32r ([v_h0 | ones | v_h1 | ones])
  aoT:      [128 d, 512] bf16 per q-tile, normalized on PSUM unload
  y:        [t, o] f32 token-major final output

Diagonal k-tiles are N-trimmed to q columns [128*rel, 512) and masked by a
single 128x128 -1e9 triangle accumulated via an identity matmul (PE, bf16).
The exp->AV dependency is software-pipelined at depth 2 so the PE never
stalls on the ACT engine.  Per-unit AllToAll is split in two [NC,128,128]
chunks (tokens 0:1024 / 1024:2048); chunk O-projections are interleaved
into the following attention work, always emitted AFTER all
collective-independent work (the local core runs ahead of the rendezvous,
and collective completions share one cumulative semaphore, so anything
emitted after a collective_compute also waits on it).
"""

import numpy as np

B, S, D, H, DK = 2, 2048, 1024, 16, 64
NC = 8
THETA = 10000.0

_COMPILED = {}


def _build():
    import concourse.bass as bass
    import concourse.tile as tile
    from concourse import bacc, mybir

    f32 = mybir.dt.float32
    f32r = mybir.dt.float32r
    bf16 = mybir.dt.bfloat16
    MUL = mybir.AluOpType.mult
    ADD = mybir.AluOpType.add
    EXP = mybir.ActivationFunctionType.Exp
    COPY = mybir.ActivationFunctionType.Copy

    nc = bacc.Bacc(num_devices=NC)

    xt_d = nc.dram_tensor("xt", [B, D, S], bf16, kind="ExternalInput")
    wqt_d = nc.dram_tensor("wqt", [D, 128], bf16, kind="ExternalInput")
    wkt_d = nc.dram_tensor("wkt", [D, 128], bf16, kind="ExternalInput")
    wvt_d = nc.dram_tensor("wvt", [D, 128], bf16, kind="ExternalInput")
    wot_d = nc.dram_tensor("wot", [D, D], bf16, kind="ExternalInput")
    cost_d = nc.dram_tensor("cost", [128, S], bf16, kind="ExternalInput")
    sinmt_d = nc.dram_tensor("sinmt", [128, S], bf16, kind="ExternalInput")
    trimask_d = nc.dram_tensor("trimask", [128, 128], bf16, kind="ExternalInput")
    ident_d = nc.dram_tensor("ident", [128, 128], f32, kind="ExternalInput")
    identb_d = nc.dram_tensor("identb", [128, 128], bf16, kind="ExternalInput")
    ones_d = nc.dram_tensor("ones", [128, 16], f32r, kind="ExternalInput")
    y_d = nc.dram_tensor("y", [B, 2, 128, D], f32, kind="ExternalOutput")

    SWAP_MASK = [(i ^ 1) for i in range(32)]

    with tile.TileContext(nc) as tc:
        with (
            tc.tile_pool(name="const", bufs=1) as constp,
            tc.tile_pool(name="xtp", bufs=4) as xtp,
            tc.tile_pool(name="qk", bufs=2) as qkp,
            tc.tile_pool(name="vp", bufs=2) as vp,
            tc.tile_pool(name="attn", bufs=3) as attnp,
            tc.tile_pool(name="ao", bufs=2) as aop,
            tc.tile_pool(name="small", bufs=1) as smallp,
            tc.tile_pool(name="rbp", bufs=2) as rbp,
            tc.tile_pool(name="rtmp", bufs=2) as rtmp,
            tc.tile_pool(name="oproj", bufs=2) as op_,
            tc.tile_pool(name="yp", bufs=2) as yp,
            tc.tile_pool(name="ps", bufs=4, space="PSUM") as psp,
            tc.tile_pool(name="dram", bufs=1, space="DRAM") as dramp,
        ):
            # ---- constant tiles ----
            cost = constp.tile([128, S], bf16)
            sinmt = constp.tile([128, S], bf16)
            trimask = constp.tile([128, 128], bf16)
            ident = constp.tile([128, 128], f32)
            identb = constp.tile([128, 128], bf16)
            ones_sb = constp.tile([128, 16], f32r)
            wq_sb = constp.tile([128, 8, 128], bf16)
            wk_sb = constp.tile([128, 8, 128], bf16)
            wv_sb = constp.tile([128, 8, 128], bf16)
            wo_sb = constp.tile([128, 8, D], bf16)

            # critical path: wq first; wk/wv follow the first xt tile so the
            # q-pass can start as early as possible
            nc.sync.dma_start(wq_sb[:], wqt_d[:, :].rearrange("(dc p) c -> p dc c", dc=8))

            warm_in = dramp.tile([NC, 64], bf16, name="warm_in")
            warm_out = dramp.tile([NC, 64], bf16, name="warm_out")
            nc.gpsimd.collective_compute(
                "AllToAll",
                mybir.AluOpType.bypass,
                replica_groups=[list(range(NC))],
                ins=[warm_in.opt()],
                outs=[warm_out.opt()],
            )
            # chunked a2a: chunk 0 = tokens [0,1024), chunk 1 = [1024,2048)
            a2a_in = [
                [dramp.tile([NC, 128, 128], bf16, name=f"a2ai{u}_{c}") for c in range(2)]
                for u in range(B)
            ]
            a2a_out = [
                [dramp.tile([NC, 128, 128], bf16, name=f"a2ao{u}_{c}") for c in range(2)]
                for u in range(B)
            ]
            recip_dram = dramp.tile([B, 8, 512], f32)

            def collective(u, c):
                nc.gpsimd.collective_compute(
                    "AllToAll",
                    mybir.AluOpType.bypass,
                    replica_groups=[list(range(NC))],
                    ins=[a2a_in[u][c].opt()],
                    outs=[a2a_out[u][c].opt()],
                )

            def o_projection(u, c):
                g = op_.tile([128, 8, 128], bf16, tag="g", name="g")
                nc.gpsimd.dma_start(g[:], a2a_out[u][c].rearrange("s p c -> p s c"))
                y_ps = psp.tile([128, 1024], f32, tag="ps", name="y_ps")
                y_sb = yp.tile([128, D], f32, tag="y", name="y_sb")
                for os_ in range(2):
                    for dc in range(8):
                        nc.tensor.matmul(
                            y_ps[:, os_ * 512:(os_ + 1) * 512],
                            g[:, dc, :],
                            wo_sb[:, dc, os_ * 512:(os_ + 1) * 512],
                            start=(dc == 0), stop=(dc == 7),
                            skip_group_check=True,
                        )
                    # unload + store each half while the other half's matmuls run
                    hsl = slice(os_ * 512, (os_ + 1) * 512)
                    nc.vector.tensor_copy(out=y_sb[:, hsl], in_=y_ps[:, hsl])
                    nc.sync.dma_start(y_d[u, c][:, hsl], y_sb[:, hsl])

            def load_xt(u, first):
                tiles = []
                for tt in range(4):
                    ts = slice(tt * 512, (tt + 1) * 512)
                    xt_sb = xtp.tile([128, 8, 512], bf16, tag="xt", name="xt_sb")
                    src = xt_d[u, :, ts].rearrange("(dc p) s -> p dc s", dc=8)
                    nc.sync.dma_start(xt_sb[:, 0:4, :], src[:, 0:4, :])
                    nc.sync.dma_start(xt_sb[:, 4:8, :], src[:, 4:8, :])
                    if first and tt == 0:
                        nc.sync.dma_start(
                            wk_sb[:], wkt_d[:, :].rearrange("(dc p) c -> p dc c", dc=8))
                        nc.sync.dma_start(
                            wv_sb[:], wvt_d[:, :].rearrange("(dc p) c -> p dc c", dc=8))
                        # consts ordered by first-use time: cos/sin quarter
                        # for RoPE tt0, then the small attention constants,
                        # then the rest of the tables
                        nc.gpsimd.dma_start(cost[:, 0:512], cost_d[:, 0:512])
                        nc.gpsimd.dma_start(sinmt[:, 0:512], sinmt_d[:, 0:512])
                        nc.gpsimd.dma_start(ident[:], ident_d[:])
                        nc.gpsimd.dma_start(identb[:], identb_d[:])
                        nc.gpsimd.dma_start(ones_sb[:], ones_d[:])
                        nc.gpsimd.dma_start(trimask[:], trimask_d[:])
                        nc.gpsimd.dma_start(cost[:, 512:1024], cost_d[:, 512:1024])
                        nc.gpsimd.dma_start(sinmt[:, 512:1024], sinmt_d[:, 512:1024])
                        nc.gpsimd.dma_start(cost[:, 1024:2048], cost_d[:, 1024:2048])
                        nc.gpsimd.dma_start(sinmt[:, 1024:2048], sinmt_d[:, 1024:2048])
                    tiles.append(xt_sb)
                return tiles

            xts = {0: load_xt(0, True)}

            for u in range(B):
                # ================= projections + RoPE =================
                qT = qkp.tile([128, S], bf16, tag="qT", name="qT")
                kT = qkp.tile([128, S], bf16, tag="kT", name="kT")
                v_sb = vp.tile([128, 16, 130], f32r, tag="v", name="v_sb")

                def proj_tt(tt):
                    ts = slice(tt * 512, (tt + 1) * 512)
                    xt_sb = xts[u][tt]
                    qk_ps = psp.tile([128, 1024], f32, tag="ps", name="qk_ps")
                    v_ps = psp.tile([128, 1024], f32, tag="ps", name="v_ps")
                    for w_sb, dst in ((wq_sb, qk_ps[:, 0:512]),
                                      (wk_sb, qk_ps[:, 512:1024]),
                                      (wv_sb, v_ps[:, 0:512])):
                        for dc in range(8):
                            nc.tensor.matmul(dst, w_sb[:, dc, :], xt_sb[:, dc, :],
                                             start=(dc == 0), stop=(dc == 7))

                    # RoPE: dst = q*cos + pairswap(q)*sinm  (bf16 out)
                    for src_, dst in ((qk_ps[:, 0:512], qT), (qk_ps[:, 512:1024], kT)):
                        qs = rtmp.tile([128, 512], f32, tag="qs", name="qs")
                        t2 = rtmp.tile([128, 512], bf16, tag="t2", name="t2")
                        nc.vector.stream_shuffle(qs[:], src_, SWAP_MASK)
                        nc.vector.tensor_tensor(out=dst[:, ts], in0=src_, in1=cost[:, ts], op=MUL)
                        nc.vector.tensor_tensor(out=t2[:], in0=qs[:], in1=sinmt[:, ts], op=MUL)
                        nc.vector.tensor_tensor(out=dst[:, ts], in0=dst[:, ts], in1=t2[:], op=ADD)

                    # v -> token-major via PE transpose; ones columns appended
                    vtmp = rtmp.tile([128, 512], f32, tag="vtmp", name="vtmp")
                    nc.vector.tensor_copy(out=vtmp[:], in_=v_ps[:, 0:512])
                    for s4 in range(4):
                        kt = tt * 4 + s4
                        tr = v_ps[:, 512 + s4 * 128: 512 + (s4 + 1) * 128]
                        nc.tensor.transpose(tr, vtmp[:, s4 * 128:(s4 + 1) * 128], ident[:])
                        dst = v_sb[:, kt, :].rearrange("p (u c) -> p u c", u=2)[:, :, 0:64]
                        src_ = tr.rearrange("p (u c) -> p u c", u=2)
                        nc.vector.tensor_copy(out=dst, in_=src_)
                    nc.vector.tensor_copy(out=v_sb[:, tt * 4:(tt + 1) * 4, 64:65],
                                          in_=ones_sb[:, tt * 4:(tt + 1) * 4])
                    nc.vector.tensor_copy(out=v_sb[:, tt * 4:(tt + 1) * 4, 129:130],
                                          in_=ones_sb[:, tt * 4:(tt + 1) * 4])

                # ================= attention =================
                def attn_qi(qi):
                    qsl = slice(qi * 512, (qi + 1) * 512)
                    outT = psp.tile([128, 1024], f32, tag="ps", name="outT")
                    n_kt = 4 * qi + 4
                    # software pipeline (depth 2): AV for tile kt is emitted
                    # after scores/exp of tile kt+2 so the PE isn't stalled
                    # on the ACT engine's exp
                    pend = []  # (at, kt, q0)

                    def flush_av(last):
                        at_, kt_, q0 = pend.pop(0)
                        for h in (0, 1):
                            nc.tensor.matmul(
                                outT[0:65, h * 512 + q0:(h + 1) * 512],
                                v_sb[:, kt_, :].rearrange("p (u c) -> p u c", u=2)[:, h, :],
                                at_[:, h * 512 + q0:(h + 1) * 512],
                                start=(kt_ == 0), stop=last,
                                skip_group_check=True,
                            )

                    for kt in range(n_kt):
                        ksl = slice(kt * 128, (kt + 1) * 128)
                        rel = kt - 4 * qi  # >=0 on the diagonal block
                        q0 = 128 * rel if rel > 0 else 0
                        sc = psp.tile([128, 1024], f32, tag="ps", name="sc")
                        at = attnp.tile([128, 1024], f32r, tag="at", name="at")
                        for h in (0, 1):
                            hp = slice(h * 64, (h + 1) * 64)
                            nc.tensor.matmul(
                                sc[:, h * 512 + q0:(h + 1) * 512],
                                kT[hp, ksl],
                                qT[hp, qsl][:, q0:512],
                                start=True, stop=(rel < 0),
                                skip_group_check=True,
                            )
                        if rel >= 0:
                            # causal triangle at q columns [q0, q0+128)
                            for h in (0, 1):
                                nc.tensor.matmul(
                                    sc[:, h * 512 + q0: h * 512 + q0 + 128],
                                    identb[:],
                                    trimask[:],
                                    start=False, stop=True,
                                    skip_group_check=True,
                                )
                        if len(pend) >= 2:
                            flush_av(False)
                        if q0 == 0:
                            nc.scalar.activation(out=at[:], in_=sc[:], func=EXP, scale=0.125)
                        else:
                            scs = sc.rearrange("p (h q) -> p h q", h=2)[:, :, q0:512]
                            ats = at.rearrange("p (h q) -> p h q", h=2)[:, :, q0:512]
                            nc.scalar.activation(out=ats, in_=scs, func=EXP, scale=0.125)
                        pend.append((at, kt, q0))
                    while len(pend) > 1:
                        flush_av(False)
                    flush_av(True)
                    if u == 1 and qi == 3:
                        # 16 matmuls for the PE while the DVE normalizes qi3;
                        # emitted before cc(1,1) exists so it only waits on A1
                        o_projection(1, 0)

                    # unload + normalize + ship this q-tile.  Denominator rows
                    # are pulled off PSUM by the ACT engine so the DVE can run
                    # the reciprocals immediately; 1/den is broadcast across
                    # each head's 64 partitions via a DRAM-roundtrip DMA.
                    aoT = aop.tile([128, 512], bf16, tag="aoT", name="aoT")
                    rb = rbp.tile([128, 512], f32, tag="rb", name="rb")
                    for h in (0, 1):
                        dent = smallp.tile([1, 512], f32, tag=f"dent{h}",
                                           name="dent", bufs=2)
                        rc = smallp.tile([1, 512], f32, tag=f"rc{h}",
                                         name="rc", bufs=2)
                        nc.scalar.activation(
                            out=dent[0:1, :], in_=outT[64:65, h * 512:(h + 1) * 512],
                            func=COPY,
                        )
                        nc.vector.reciprocal_approx_fast(out=rc[0:1, :], in_=dent[0:1, :])
                        nc.sync.dma_start(
                            recip_dram[u, h * 4 + qi: h * 4 + qi + 1, :], rc[0:1, :]
                        )
                        nc.gpsimd.dma_start(
                            rb[h * 64:(h + 1) * 64, :],
                            recip_dram[u, h * 4 + qi: h * 4 + qi + 1, :].to_broadcast([64, 512]),
                        )
                    for h in (0, 1):
                        nc.vector.tensor_copy(
                            out=aoT[h * 64:(h + 1) * 64, :],
                            in_=outT[0:64, h * 512:(h + 1) * 512],
                        )
                    nc.vector.tensor_tensor(out=aoT[:], in0=aoT[:], in1=rb[:], op=MUL)
                    # ship: chunk c = qi//2, slots 4*(qi%2) .. +4
                    c = qi // 2
                    j0 = 4 * (qi % 2)
                    nc.sync.dma_start(
                        a2a_in[u][c][j0:j0 + 4].rearrange("s p c -> p s c"),
                        aoT[:].rearrange("p (s c) -> p s c", s=4),
                    )
                    if u == 0 and qi == 1:
                        collective(0, 0)
                    elif u == 0 and qi == 3:
                        collective(0, 1)
                        # O-projection weights: off the startup critical path
                        for dc in range(8):
                            nc.gpsimd.dma_start(
                                wo_sb[:, dc, :], wot_d[dc * 128:(dc + 1) * 128, :]
                            )
                    elif u == 1 and qi == 1:
                        o_projection(0, 0)
                        collective(1, 0)
                    elif u == 1 and qi == 2:
                        o_projection(0, 1)
                    elif u == 1 and qi == 3:
                        collective(1, 1)

                # attention q-tile i only needs projection tiles 0..i, so
                # interleave: attention work keeps the PE busy while later
                # xt tiles stream in from HBM
                proj_tt(0)
                proj_tt(1)
                attn_qi(0)
                proj_tt(2)
                attn_qi(1)
                proj_tt(3)
                if u == 0:
                    # prefetch unit 1's activations during unit 0's attention
                    xts[1] = load_xt(1, False)
                attn_qi(2)
                attn_qi(3)

            o_projection(1, 1)

    nc.compile()
    return nc


def _host_inputs(x, wq, wk, wv, wo):
    import ml_dtypes

    bf = ml_dtypes.bfloat16
    xt = np.ascontiguousarray(x.transpose(0, 2, 1)).astype(bf)
    wot = np.ascontiguousarray(wo.T).astype(bf)

    p = np.arange(128)
    invf = THETA ** (-2.0 * ((p % 64) // 2) / 64.0)
    ang = invf[:, None] * np.arange(S)[None, :]
    cost = np.cos(ang).astype(bf)
    sinmt = (np.sin(ang) * np.where(p % 2 == 0, -1.0, 1.0)[:, None]).astype(bf)

    i = np.arange(128)[:, None]
    j = np.arange(128)[None, :]
    # additive causal triangle: 0 where allowed (j >= i), -1e9 where masked
    trimask = np.where(j >= i, 0.0, -1e9).astype(bf)
    ident = np.eye(128, dtype=np.float32)

    in_maps = []
    for c in range(NC):
        sl = slice(c * 128, (c + 1) * 128)
        in_maps.append({
            "xt": xt,
            "wqt": np.ascontiguousarray(wq[sl, :].T).astype(bf),
            "wkt": np.ascontiguousarray(wk[sl, :].T).astype(bf),
            "wvt": np.ascontiguousarray(wv[sl, :].T).astype(bf),
            "wot": wot,
            "cost": cost,
            "sinmt": sinmt,
            "trimask": trimask,
            "ident": ident,
            "identb": ident.astype(bf),
            "ones": np.ones((128, 16), np.float32),
        })
    return in_maps


def kernel(x, wq, wk, wv, wo, _trace=False):
    from concourse.bass_utils import run_bass_kernel_spmd

    if "nc" not in _COMPILED:
        _COMPILED["nc"] = _build()
    nc = _COMPILED["nc"]

    in_maps = _host_inputs(
        np.asarray(x, np.float32), np.asarray(wq, np.float32),
        np.asarray(wk, np.float32), np.asarray(wv, np.float32),
        np.asarray(wo, np.float32),
    )
    res = run_bass_kernel_spmd(nc, in_maps, core_ids=list(range(NC)), trace=_trace)
    _COMPILED["last_result"] = res

    y = np.zeros((B, S, D), np.float32)
    for c in range(NC):
        yc = res.results[c]["y"]  # [B, 2, 128, D]
        for u in range(B):
            y[u, 128 * c: 128 * c + 128, :] = yc[u, 0]
            y[u, 1024 + 128 * c: 1024 + 128 * c + 128, :] = yc[u, 1]
    return y
